# revision 84
# baseline (speedup 1.0000x reference)
"""GCNCritic forward kernel for Trainium2 (Bass/Tile), 8-core data-parallel.

Math collapse: the reference GCN runs on fully-connected 16-node graphs with
self-loops, so each GCN layer's output is constant across the 16 nodes of a
sample.  The two GCN layers + global_mean_pool reduce to per-sample matmuls
on the per-sample mean of x = relu(obs @ W_pre + b_pre):

    xm = mean_nodes(relu(obs @ W_pre + b_pre))            # [B, HID]
    x1 = relu(xm @ W_gcn0 + b_gcn0)                       # [B, HID]
    x2 = relu(x1 @ W_gcn1 + b_gcn1)                       # [B, HID]
    g  = relu(x2 @ W_post + b_post)                       # [B, GE]
    gz = g @ W1[:GE] + b1                                 # [B, F1]
    loc = relu(obs @ W_loc + b_loc)                       # [B*n, LE]
    z1 = relu(loc @ W1[GE:] + gz[sample])                 # [B*n, F1]
    z2 = relu(z1 @ W2 + b2)                               # [B*n, F2]
    q  = z2 @ W3 (+ b3 on host)                           # [B*n, 8]

Sharding: batch (2048 samples) split across 8 NeuronCores, 256 samples
(4096 nodes) per core; weights replicated.  Activations are kept
feature-on-partitions; every weight is consumed as lhsT in natural [K, M]
layout, so the device program contains no transposes.  All matmul operands
are bf16 (same PE rate as fp32r here, half the DMA/SBUF traffic, rel-err
~6e-3 vs the 2e-2 gate); PSUM accumulation stays fp32.

Measured (TimelineSim cost model): 61.2us/core vs 73.9us for the fp32r
predecessor; hardware rel-err 5.9e-3 vs the 2e-2 gate.

Schedule (the PE is the bottleneck at ~46us of real matmul; everything else
is arranged so it never waits — note GPSIMD cannot read PSUM, so all PSUM
evacuation goes through ACT/DVE and Pool only gets SBUF->SBUF work):
- junk matmuls on a zeroed tile burn the PE p-state ramp under the
  initial DMA latency.
- phase A is split: an x-pass (x matmul -> ACT relu -> DVE group-sum) runs
  first so the serial B chain (xm->x1->x2->g->gz, per-sample) can start
  after only 4 tiles; the loc-pass tiles are deferred and woven into the
  big phase-C loop, which only needs locT(t) just before z1(t).
- B is split into mini-chains of graded width — tile 0 alone (so gz(0)
  lands as early as possible), tiles 1-3 as one chain, tiles 4-7 as one
  chain: the early chain unblocks phase C fast while the coarse late
  chains minimize weave items and their queue traffic, which pace
  iterations 0-3.  Every hop is emitted between dense matmul chunks so
  its evac latency hides.
  The DMA order, engine assignment (z2 evacs on ACT, z1 adds+relus on
  DVE) and PSUM split (zps=4/aps=3/qps=1) are a jointly-tuned equilibrium:
  single-variable perturbations in any direction measured slower.
- q is computed ROW-major (z2 k-tiles as lhsT, W3 as an 8-wide rhs): ~1k
  PE cycles instead of 16k, and the output lands as [128, 256] fp32 so the
  final DMA is a fraction of a us (vs 3us+ for an [8, 4096] layout).
  b3 is added host-side (it is the last op of the network).
- PSUM banks are partitioned by tag: 4 for A/B, 3 rotating for z1/z2,
  1 for q (shared with the warmup junk).
"""

import numpy as np
import ml_dtypes

import concourse.bass as bass
import concourse.mybir as mybir
import concourse.tile as tile
from concourse.bass import ts
from concourse.bass_utils import run_bass_kernel_spmd

OBS = 128
N_AGENT = 16
HID = 128
GE = 256
LE = 256
F1 = 512
F2 = 512
NA = 8
B = 2048
NCORES = 8
BS = B // NCORES            # 256 samples per core
R = BS * N_AGENT            # 4096 rows (nodes) per core
RT = 512                    # rows per tile
NT = R // RT                # 8 row tiles
SPT = RT // N_AGENT         # 32 samples per row tile
RB = RT // 128              # 4 row blocks (128 rows) per tile

F32 = mybir.dt.float32
BF16 = mybir.dt.bfloat16
RELU = mybir.ActivationFunctionType.Relu
ADD = mybir.AluOpType.add
MAX = mybir.AluOpType.max

# bf16 weight pack column layout ([K,M] tiles of [128,128] at OFF + k*M +
# m*128).  The fp32 biases ride bit-packed in the first 32 bf16 columns and
# are read back on-device via a size-changing bitcast.
BOFF = 0             # 16 fp32 bias cols as 32 bf16 cols
W_PRE = 32           # [128, 128]
W_LOC = 160          # [128, 256]
W_G0 = 416           # [128, 128]
W_G1 = 544           # [128, 128]
W_POST = 704         # [128, 256]
W1A = 960            # [256, 512] -> 2 k-tiles of 512
W1B = 1984           # [256, 512]
W2O = 3008           # [512, 512] -> 4 k-tiles of 512
W3O = 5056           # [512, 8]   -> 4 k-tiles of 8
BCOLS = 5088

# bias column indices (fp32 units within the bit-packed block)
B_PRE = 0
B_G0 = 1
B_G1 = 2
B_POST = 3           # 2 cols
B_LOC = 5            # 2 cols
B_B1 = 7             # 4 cols
B_B2 = 11            # 4 cols
SCOLS = 16

NJUNK = 10            # PE warmup matmuls to burn the p-state ramp


def _pack_weights(i):
    pk = np.zeros((128, BCOLS), np.float32)
    pk[:, W_PRE:W_PRE + 128] = i["W_pre"]
    pk[:, W_LOC:W_LOC + 256] = i["W_loc"]
    pk[:, W_G0:W_G0 + 128] = i["W_gcn"][0] / N_AGENT
    pk[:, W_G1:W_G1 + 128] = i["W_gcn"][1]
    pk[:, W_POST:W_POST + 256] = i["W_post"]
    for k in range(2):
        pk[:, W1A + k * F1:W1A + (k + 1) * F1] = i["W1"][k * 128:(k + 1) * 128]
        pk[:, W1B + k * F1:W1B + (k + 1) * F1] = i["W1"][(2 + k) * 128:(3 + k) * 128]
    for k in range(4):
        pk[:, W2O + k * F2:W2O + (k + 1) * F2] = i["W2"][k * 128:(k + 1) * 128]
        pk[:, W3O + k * NA:W3O + (k + 1) * NA] = i["W3"][k * 128:(k + 1) * 128]
    pb = pk.astype(ml_dtypes.bfloat16)
    # biases: exact fp32 bits split across pairs of bf16 columns
    pb[:, BOFF:BOFF + 2 * SCOLS] = _pack_small(i).view(ml_dtypes.bfloat16)
    return pb


def _pack_small(i):
    sm = np.zeros((128, SCOLS), np.float32)
    sm[:, B_PRE] = i["b_pre"]
    sm[:, B_G0] = i["b_gcn"][0]
    sm[:, B_G1] = i["b_gcn"][1]
    sm[:, B_POST:B_POST + 2] = i["b_post"].reshape(2, 128).T
    sm[:, B_LOC:B_LOC + 2] = i["b_loc"].reshape(2, 128).T
    sm[:, B_B1:B_B1 + 4] = i["b1"].reshape(4, 128).T
    sm[:, B_B2:B_B2 + 4] = i["b2"].reshape(4, 128).T
    return sm


def _build():
    nc = bass.Bass("TRN2", target_bir_lowering=False, debug=False)

    obs_h = nc.dram_tensor("obs", [OBS, R], BF16, kind="ExternalInput")
    wpack_h = nc.dram_tensor("wpack", [128, BCOLS], BF16, kind="ExternalInput")
    out_h = nc.dram_tensor("out", [128, NT * RB * NA], F32, kind="ExternalOutput")

    with tile.TileContext(nc) as tc:
        with (
            tc.tile_pool(name="consts", bufs=1) as consts,
            tc.tile_pool(name="persist", bufs=1) as persist,
            tc.tile_pool(name="work", bufs=2) as work,
            tc.tile_pool(name="zwork", bufs=2) as zwork,
            tc.tile_pool(name="ps", bufs=4, space="PSUM") as psp,
        ):
            def atile():
                return psp.tile([128, 512], F32, tag="aps", name="aps", bufs=3)

            def ztile():
                return psp.tile([128, 512], F32, tag="zps", name="zps", bufs=4)

            # ---- PE warmup: junk matmuls with no data deps burn the
            # p-state ramp while the first DMAs are in flight ----
            warm = consts.tile([128, 512], BF16, tag="warm")
            nc.vector.memset(warm, 0.0)
            wps = psp.tile([128, 512], F32, tag="qps", name="wps", bufs=1)
            for _ in range(NJUNK):
                nc.tensor.matmul(
                    wps[:, :256], warm[:, :128], warm[:, :256],
                    start=True, stop=True,
                )

            # ---- constants + obs, in consumer order ----
            wp = consts.tile([128, BCOLS], BF16, tag="wp")
            obsb = consts.tile([128, R], BF16, tag="obsb")
            nc.sync.dma_start(obsb[:, :RT], obs_h[:, :RT])
            nc.sync.dma_start(wp[:, :W_G0], wpack_h[:, :W_G0])
            nc.sync.dma_start(obsb[:, RT:4 * RT], obs_h[:, RT:4 * RT])
            nc.sync.dma_start(wp[:, W_G0:W1B], wpack_h[:, W_G0:W1B])
            nc.sync.dma_start(wp[:, W1B:W2O], wpack_h[:, W1B:W2O])
            nc.sync.dma_start(wp[:, W2O:W3O], wpack_h[:, W2O:W3O])
            nc.sync.dma_start(obsb[:, 4 * RT:], obs_h[:, 4 * RT:])
            nc.sync.dma_start(wp[:, W3O:], wpack_h[:, W3O:])

            def bias(c0):
                return wp[:, BOFF + 2 * c0:BOFF + 2 * c0 + 2].bitcast(F32)

            # ---- persistent activations ----
            locT = persist.tile([128, 2, NT, RT], BF16, tag="locT")
            xsum = persist.tile([128, BS], BF16, tag="xsum")
            gz = persist.tile([128, 4, BS], F32, tag="gz")
            qacc = persist.tile([128, NT, RB, NA], F32, tag="qacc")

            # ---- phase A, x half: obs -> x^T -> per-sample sums ----
            _xps = {}

            def x_mm(t):
                x_ps = atile()
                nc.tensor.matmul(
                    x_ps, wp[:, W_PRE:W_PRE + 128], obsb[:, ts(t, RT)],
                    start=True, stop=True,
                )
                _xps[t] = x_ps

            def x_fin(t):
                x_ps = _xps.pop(t)
                xT = work.tile([128, RT], BF16, tag="xT", name="xT")
                nc.scalar.activation(xT, x_ps, RELU, bias=bias(B_PRE))
                with nc.allow_low_precision(reason="bf16 16-elem sum"):
                    nc.vector.tensor_reduce(
                        xsum[:, ts(t, SPT)],
                        xT.rearrange("p (s k) -> p s k", k=N_AGENT),
                        axis=mybir.AxisListType.X,
                        op=mybir.AluOpType.add,
                    )

            def x_pass(t):
                x_mm(t)
                x_fin(t)

            # ---- phase A, loc half (deferred; woven into phase C) ----
            def loc_pass(t):
                for m in range(2):
                    l_ps = ztile()
                    nc.tensor.matmul(
                        l_ps, wp[:, W_LOC + m * 128:W_LOC + (m + 1) * 128],
                        obsb[:, ts(t, RT)], start=True, stop=True,
                    )
                    nc.scalar.activation(
                        locT[:, m, t, :], l_ps, RELU, bias=bias(B_LOC + m)
                    )

            # ---- phase B: per-sample chain, split into 8 per-tile
            # mini-chains (32 samples each) so chain t only needs the
            # x-reduction of tile t; hops are woven between dense PE work ----
            def b_evac(t, out, ps, b):
                if t == 0:
                    nc.vector.tensor_scalar(
                        out, ps, b, 0.0, op0=ADD, op1=MAX
                    )
                else:
                    nc.scalar.activation(out, ps, RELU, bias=b)

            def B_x1(c):
                t0, nt = CHAINS[c]
                W = nt * SPT
                S = slice(t0 * SPT, t0 * SPT + W)
                x1_ps = atile()
                nc.tensor.matmul(
                    x1_ps[:, :W], wp[:, W_G0:W_G0 + 128], xsum[:, S],
                    start=True, stop=True,
                )
                x1 = work.tile([128, 128], BF16, tag="x1", name="x1", bufs=4)
                b_evac(c, x1[:, :W], x1_ps[:, :W], bias(B_G0))
                return x1

            def B_x2(c, x1):
                t0, nt = CHAINS[c]
                W = nt * SPT
                x2_ps = atile()
                nc.tensor.matmul(
                    x2_ps[:, :W], wp[:, W_G1:W_G1 + 128], x1[:, :W],
                    start=True, stop=True,
                )
                x2 = work.tile([128, 128], BF16, tag="x2", name="x2", bufs=4)
                b_evac(c, x2[:, :W], x2_ps[:, :W], bias(B_G1))
                return x2

            def B_g(c, x2):
                t0, nt = CHAINS[c]
                W = nt * SPT
                g = work.tile([128, 2, 128], BF16, tag="g", name="g", bufs=4)
                for m in range(2):
                    g_ps = atile()
                    nc.tensor.matmul(
                        g_ps[:, :W],
                        wp[:, W_POST + m * 128:W_POST + (m + 1) * 128],
                        x2[:, :W], start=True, stop=True,
                    )
                    b_evac(c, g[:, m, :W], g_ps[:, :W], bias(B_POST + m))
                return g

            def B_gz(c, g):
                t0, nt = CHAINS[c]
                W = nt * SPT
                S = slice(t0 * SPT, t0 * SPT + W)
                for m in range(4):
                    gz_ps = atile()
                    for k in range(2):
                        nc.tensor.matmul(
                            gz_ps[:, :W],
                            wp[:, W1A + k * F1 + m * 128:W1A + k * F1 + (m + 1) * 128],
                            g[:, k, :W], start=(k == 0), stop=(k == 1),
                        )
                    nc.vector.tensor_scalar_add(
                        gz[:, m, S], gz_ps[:, :W], bias(B_B1 + m)
                    )

            # ---- phase C ----
            def z1_mblock(t, m, z1):
                z_ps = ztile()
                for k in range(2):
                    nc.tensor.matmul(
                        z_ps,
                        wp[:, W1B + k * F1 + m * 128:W1B + k * F1 + (m + 1) * 128],
                        locT[:, k, t, :], start=(k == 0), stop=(k == 1),
                    )
                gzb = gz[:, m, ts(t, SPT)][:, :, None].to_broadcast(
                    [128, SPT, N_AGENT]
                )
                nc.vector.tensor_add(
                    z1[:, m, :].rearrange("p (s k) -> p s k", k=N_AGENT),
                    z_ps.rearrange("p (s k) -> p s k", k=N_AGENT),
                    gzb,
                )
                nc.vector.tensor_scalar_max(z1[:, m, :], z1[:, m, :], 0.0)

            def z1_tile_alloc():
                return zwork.tile([128, 4, RT], BF16, tag="z1", bufs=2, name="z1")

            def z2_mblock(t, m, z1, z2, last=False):
                z_ps = ztile()
                for k in range(4):
                    nc.tensor.matmul(
                        z_ps,
                        wp[:, W2O + k * F2 + m * 128:W2O + k * F2 + (m + 1) * 128],
                        z1[:, k, :], start=(k == 0), stop=(k == 3),
                    )
                if last and m == 3:
                    # shorten the tail: split the final evac across two
                    # engines so the last q matmuls fire sooner
                    nc.scalar.activation(
                        z2[:, m, :RT // 2], z_ps[:, :RT // 2], RELU,
                        bias=bias(B_B2 + m),
                    )
                    nc.vector.tensor_scalar(
                        z2[:, m, RT // 2:], z_ps[:, RT // 2:], bias(B_B2 + m),
                        0.0, op0=ADD, op1=MAX,
                    )
                else:
                    nc.scalar.activation(
                        z2[:, m, :], z_ps, RELU, bias=bias(B_B2 + m)
                    )

            def q_mms(t, z2, kk):
                # q row-major: z2 k-tiles as lhsT, W3 as 8-wide rhs; all 16
                # matmuls accumulate into one PSUM bank (4 disjoint row-block
                # slices); zero-on-first-write is armed once for the bank.
                if kk == 0:
                    self_q = psp.tile(
                        [128, RB, NA], F32, tag="qps", name="qps", bufs=1
                    )
                    _QPS[t] = self_q
                q_ps = _QPS[t]
                for rb in range(RB):
                    nc.tensor.matmul(
                        q_ps[:, rb, :],
                        z2[:, kk, rb * 128:(rb + 1) * 128],
                        wp[:, W3O + kk * NA:W3O + (kk + 1) * NA],
                        start=(kk == 0 and rb == 0),
                        stop=(kk == 3 and rb == RB - 1),
                        skip_group_check=True,
                    )

            def q_finish(t):
                q_ps = _QPS.pop(t)
                nc.vector.tensor_scalar_add(qacc[:, t], q_ps, 0.0)
                if t == NT - 2:
                    nc.sync.dma_start(
                        out_h[:, :(NT - 1) * RB * NA], qacc[:, :NT - 1]
                    )
                elif t == NT - 1:
                    nc.sync.dma_start(
                        out_h[:, (NT - 1) * RB * NA:], qacc[:, NT - 1]
                    )

            _QPS = {}
            CHAINS = [(0, 1), (1, 3), (4, 4), (0, 0)]

            # ---- B mini-chain step driver ----
            _bst = {}

            _bcnt = {}

            def B_next(t):
                s = _bcnt.get(t, 0)
                if s == 0:
                    _bst[t] = B_x1(t)
                elif s == 1:
                    _bst[t] = B_x2(t, _bst[t])
                elif s == 2:
                    _bst[t] = B_g(t, _bst[t])
                elif s == 3:
                    B_gz(t, _bst[t])
                    _bst[t] = None
                _bcnt[t] = s + 1

            # ---- emission schedule ----
            # interleave so each engine queue sees latency-critical items in
            # dependency order: DVE = reduces + B0 evacs + TT/relu, ACT = xT
            # + B1..7 evacs + z2, Pool = loc + gz(1..7) + z2.
            x_mm(0)
            x_mm(1)
            x_fin(0)
            loc_pass(0)
            x_mm(2)
            x_fin(1)
            loc_pass(1)
            B_next(0)                 # x1(0)
            x_mm(3)
            x_fin(2)
            B_next(0)                 # x2(0)
            x_fin(3)
            B_next(0)                 # g(0)
            B_next(1)                 # x1(1)
            B_next(0)                 # gz(0)
            B_next(1)                 # x2(1)
            loc_pass(2)
            B_next(1)                 # g(1)
            loc_pass(3)

            z1cur = z1_tile_alloc()
            for m in range(4):
                z1_mblock(0, m, z1cur)
            B_next(1)                 # gz(1)

            def BN(t):
                return lambda: B_next(t)

            weave = [
                [lambda: x_mm(4)],                             # i0-A
                [lambda: x_fin(4), lambda: x_mm(5)],           # i0-B
                [lambda: x_fin(5), lambda: x_mm(6)],           # i0-C
                [BN(2), lambda: x_fin(6)],                     # i0-D
                [BN(2), lambda: x_mm(7)],                      # i1-A
                [BN(2), lambda: x_fin(7)],                     # i1-B
                [BN(2), lambda: loc_pass(4)],                  # i1-C
                [],                                            # i1-D
                [],                                            # i2-A
                [lambda: loc_pass(5)],                         # i2-B
                [], [],
                [lambda: loc_pass(6)],                         # i3-A
                [], [], [],
                [lambda: loc_pass(7)],                         # i4-A
                [], [], [],
                [], [], [], [],
                [], [], [], [],
            ]

            def weave_one():
                if weave:
                    for fn in weave.pop(0):
                        fn()

            for t in range(NT):
                z1nxt = z1_tile_alloc() if t + 1 < NT else None
                z2 = zwork.tile([128, 4, RT], BF16, tag="z2", bufs=2, name="z2")
                for m in range(4):
                    z2_mblock(t, m, z1cur, z2, last=(t == NT - 1))
                    if z1nxt is not None:
                        z1_mblock(t + 1, m, z1nxt)
                    if m == 1:
                        weave_one()
                q_mms(t, z2, 0)
                q_mms(t, z2, 1)
                weave_one()
                q_mms(t, z2, 2)
                weave_one()
                q_mms(t, z2, 3)
                q_finish(t)
                weave_one()
                z1cur = z1nxt

    _split_waits(nc)
    return nc


def _split_waits(nc):
    # walrus accepts only one sync-wait per instruction in this build; move
    # extra waits onto same-engine sequencer nops placed immediately before
    # the instruction (program order on the engine's queue, so semantics are
    # identical).
    for blk in nc.m.functions[0].blocks:
        new = []
        for inst in blk.instructions:
            if inst.sync_info is not None:
                w = list(inst.sync_info.on_wait)
                if len(w) > 1:
                    for wx in w[:-1]:
                        new.append(
                            mybir.InstNoOp(
                                name=nc.get_next_instruction_name(),
                                engine=inst.engine,
                                sync_info=mybir.SyncInfo(
                                    on_wait=[wx], on_update=[]
                                ),
                                bass_nofuse=True,
                            )
                        )
                    inst.sync_info.on_wait = [w[-1]]
            new.append(inst)
        blk.instructions[:] = new


_CACHE = {}


def _get_nc():
    if "nc" not in _CACHE:
        _CACHE["nc"] = _build()
    return _CACHE["nc"]


def kernel(trace=False, **inputs):
    obs_j = np.ascontiguousarray(np.asarray(inputs["obs_j"], dtype=np.float32))
    np_in = {
        k: np.asarray(v, dtype=np.float32)
        for k, v in inputs.items()
        if k != "obs_j"
    }
    pack = np.ascontiguousarray(_pack_weights(np_in))
    b3 = np_in["b3"]
    nc = _get_nc()
    in_maps = []
    for c in range(NCORES):
        ob = obs_j[c * BS:(c + 1) * BS].reshape(R, OBS).T
        in_maps.append({
            "obs": np.ascontiguousarray(ob.astype(ml_dtypes.bfloat16)),
            "wpack": pack,
        })
    res = run_bass_kernel_spmd(
        nc, in_maps, core_ids=list(range(NCORES)), trace=trace
    )
    outs = []
    for c in range(NCORES):
        o = res.results[c]["out"].reshape(128, NT, RB, NA)
        # row r = t*512 + rb*128 + p  ->  q[r, a] = o[p, t, rb, a]
        outs.append(np.transpose(o, (1, 2, 0, 3)).reshape(R, NA))
    q = np.concatenate(outs, axis=0) + b3
    q = np.ascontiguousarray(q).reshape(B, N_AGENT, NA)
    if trace:
        return q, res
    return q
